# revision 4
# baseline (speedup 1.0000x reference)
"""Sharded top-1 KNN (retrieval) on 8 TRN2 NeuronCores via Bass/Tile.

v4 strategy (hardcoded for x[2048,24,16], X_train[65536,384], Y_train[65536,24,1]):
  - Shard X_train rows across 8 cores (8192 rows each).
  - fp8(e4m3) GEMM: cross = x.t with TensorE DoubleRow matmuls (K=256 in one
    MM at ~1 col/cycle) plus a plain fp8 MM for the K=128 tail -> ~1.4x the
    bf16 matmul rate.  Host pre-quantizes x -> [384,2048] fp8 and each
    (tt-sort-permuted) X_train shard -> [384,8192] fp8.
  - Drain: each m-tile's 16 psum chunks (chunk = q*4+j) are max-folded 4->1
    over q into a [128,2048] bf16 fold-4 row using wide ops only: ScalarE
    copies 12 chunks (3 x [128,2048]-ish ACT copies), VectorE merges the
    last 4 psum chunks against one copy and folds the tree (4 wide TTs).
    GpSimd/DMA do no psum work (no PSUM port).  Fold-4 rows DMA to HBM.
  - No bias / top-k on device: the HOST applies the shared -||t||^2/2 bias
    per fold-4 column (4 tt-adjacent rows), takes top-10 columns per core
    (exact, tie-free), expands 4 rows per column, and refines exact
    distances in fp64 (ties -> smallest global index, matching jnp.argmin).
  - Max-pooling cannot hurt candidate recall: the true NN's pooled column
    value >= its own score (measured fold-4 pooled rank <= 2 vs 10 kept).
"""

import os
import sys

import numpy as np

for _p in ("/opt/trn_rl_repo",):
    if os.path.isdir(_p) and _p not in sys.path:
        sys.path.insert(0, _p)

import ml_dtypes  # noqa: E402

B, T, F = 2048, 24, 16
D = T * F  # 384
N = 65536
NCORES = 8
NS = N // NCORES  # 8192 rows per core
MT = B // 128  # 16 query tiles
NCHUNK = 512
NT = NS // NCHUNK  # 16 train chunks per core
NPOOL = 4 * NCHUNK  # 2048 fold-4 pooled positions (j*512 + col)
TOPK = 10  # host-side; fold-4 pooled rank of true NN measured <= 2
NBLK = 8  # X DMA split into 8 column blocks of 1024 for early start
BLKW = NS // NBLK  # 1024

_BF16 = ml_dtypes.bfloat16
_FP8 = ml_dtypes.float8_e4m3


def build_nc(b=B, ns=NS):
    """Build the per-core Bass program (SPMD: same program, per-core inputs)."""
    import concourse.tile as tile
    from concourse import bacc, mybir

    fp8 = mybir.dt.float8e4
    bf16 = mybir.dt.bfloat16
    f32 = mybir.dt.float32
    mx = mybir.AluOpType.max
    DR = mybir.MatmulPerfMode.DoubleRow

    nc = bacc.Bacc(None, target_bir_lowering=False)
    xdr = nc.dram_tensor("xdr", [256, b], fp8, kind="ExternalInput")
    xtl = nc.dram_tensor("xtl", [128, b], fp8, kind="ExternalInput")
    Xdr = nc.dram_tensor("Xdr", [256, ns], fp8, kind="ExternalInput")
    Xtl = nc.dram_tensor("Xtl", [128, ns], fp8, kind="ExternalInput")
    pooled = nc.dram_tensor("pooled", [b, NPOOL], bf16, kind="ExternalOutput")

    with tile.TileContext(nc) as tc:
        with (
            tc.tile_pool(name="wpool", bufs=1) as wpool,
            tc.tile_pool(name="ppool", bufs=1, space="PSUM") as ppool,
            tc.tile_pool(name="spool", bufs=2) as spool,
        ):
            # query weights: DR layout [128, 2, b] (k 0..255) + tail [128, b]
            xw = wpool.tile([128, 2, b], fp8, name="xw", tag="xw")
            nc.sync.dma_start(xw[:, 0, :], xdr[0:128, :])
            nc.sync.dma_start(xw[:, 1, :], xdr[128:256, :])
            xt = wpool.tile([128, b], fp8, name="xt", tag="xt")
            nc.sync.dma_start(xt[:], xtl[:, :])
            # train blocks: 8 col-blocks of 1024, each DR [128,2,1024] + tail
            Xd_b = []
            Xt_b = []
            for blk in range(NBLK):
                cs = slice(blk * BLKW, (blk + 1) * BLKW)
                td = wpool.tile([128, 2, BLKW], fp8, name="Xd", tag=f"Xd{blk}")
                nc.sync.dma_start(td[:, 0, :], Xdr[0:128, cs])
                nc.sync.dma_start(td[:, 1, :], Xdr[128:256, cs])
                Xd_b.append(td)
                tt_ = wpool.tile([128, BLKW], fp8, name="Xt", tag=f"Xt{blk}")
                nc.sync.dma_start(tt_[:], Xtl[:, cs])
                Xt_b.append(tt_)

            def mm_pair(out_ap, m, c):
                ms = slice(m * 128, (m + 1) * 128)
                blk, lo = c // 2, (c % 2) * NCHUNK
                nc.tensor.matmul(
                    out_ap,
                    xw[:, :, ms],
                    Xd_b[blk][:, :, lo : lo + NCHUNK],
                    start=True,
                    stop=False,
                    perf_mode=DR,
                )
                nc.tensor.matmul(
                    out_ap,
                    xt[:, ms],
                    Xt_b[blk][:, lo : lo + NCHUNK],
                    start=False,
                    stop=True,
                )

            for m in range(MT):
                ms = slice(m * 128, (m + 1) * 128)
                # psum: 4 tiles x 2 banks; per group g tile q holds chunks
                # 8g+2q, 8g+2q+1.  chunk c = q*4+j (j = fold lane); the
                # fold-4 column j*512+col pools chunks {j, 4+j, 8+j, 12+j}.
                # g0 drains via 4 ScalarE copies; g1 via 4 VectorE TT-maxes
                # against those copies (each frees its psum tile ~1.6us
                # before PE rotates back onto it), then 2 half-merges.
                cps = None
                for g in range(2):
                    P = [
                        ppool.tile([128, 2, NCHUNK], f32, name="P", tag=f"P{q}")
                        for q in range(4)
                    ]
                    for q in range(4):
                        for j in range(2):
                            mm_pair(P[q][:, j, :], m, 8 * g + 2 * q + j)
                    if g == 0:
                        cps = [
                            spool.tile([128, 2, NCHUNK], bf16, name="c", tag=f"c{q}")
                            for q in range(4)
                        ]
                        for q in range(4):
                            nc.scalar.copy(cps[q][:], P[q][:])
                    else:
                        wAC = spool.tile([128, 4, NCHUNK], bf16, name="wAC")
                        wBD = spool.tile([128, 4, NCHUNK], bf16, name="wBD")
                        nc.vector.tensor_tensor(
                            wAC[:, 0:2, :], P[0][:], cps[0][:], op=mx
                        )
                        nc.vector.tensor_tensor(
                            wAC[:, 2:4, :], P[1][:], cps[1][:], op=mx
                        )
                        nc.vector.tensor_tensor(
                            wBD[:, 0:2, :], P[2][:], cps[2][:], op=mx
                        )
                        nc.vector.tensor_tensor(
                            wBD[:, 2:4, :], P[3][:], cps[3][:], op=mx
                        )
                        tv = spool.tile([128, NPOOL], bf16, name="tv")
                        nc.vector.tensor_tensor(
                            tv[:, 0:1024], wAC[:, 0:2, :], wBD[:, 0:2, :], op=mx
                        )
                        nc.vector.tensor_tensor(
                            tv[:, 1024:2048], wAC[:, 2:4, :], wBD[:, 2:4, :], op=mx
                        )
                        nc.sync.dma_start(pooled[ms, 0:1024], tv[:, 0:1024])
                        nc.sync.dma_start(pooled[ms, 1024:2048], tv[:, 1024:2048])
    nc.finalize()  # Bacc register allocation; walrus rejects unfinalized BIR
    return nc


_NC = None


def _get_nc():
    global _NC
    if _NC is None:
        _NC = build_nc()
    return _NC


def _shard_perm(tt, ns):
    """Device row n = chunk*512+col, chunk = q*4+j, holds sorted rank
    (col*4+j)*4+q: the 4 rows folded into fold-4 column (j, col) are
    tt-adjacent so one shared bias serves all 4."""
    order = np.argsort(tt, kind="stable")
    n = np.arange(ns)
    chunk, col = n // NCHUNK, n % NCHUNK
    j, q = chunk % 4, chunk // 4
    rank = (col * 4 + j) * 4 + q
    return order[rank]


def _prep_in_maps(xf, X_train):
    xq = np.ascontiguousarray(xf.T).astype(_FP8)  # [384, 2048] fp8
    in_maps = []
    perms = []
    biases = []
    for c in range(NCORES):
        Xs = X_train[c * NS : (c + 1) * NS]
        tt = (Xs.astype(np.float64) ** 2).sum(axis=1)
        perm = _shard_perm(tt, NS)
        perms.append(perm)
        XT = np.ascontiguousarray(Xs[perm].T).astype(_FP8)  # [384, 8192]
        tts = np.sort(tt, kind="stable")
        # bias for fold-4 column j*512+col = mean tt/2 of ranks (col*4+j)*4..+4
        bias = (tts.reshape(NCHUNK, 4, 4).mean(axis=2) * 0.5).T.reshape(NPOOL)
        biases.append(bias.astype(np.float32))
        in_maps.append(
            {
                "xdr": np.ascontiguousarray(xq[0:256]),
                "xtl": np.ascontiguousarray(xq[256:384]),
                "Xdr": np.ascontiguousarray(XT[0:256]),
                "Xtl": np.ascontiguousarray(XT[256:384]),
            }
        )
    return in_maps, perms, biases


def _refine(xf, X_train, Y_train, cand):
    """cand: [B, C] global candidate rows.  fp64 exact distances, ties ->
    smallest global index (matches jnp.argmin first-of-min)."""
    b = cand.shape[0]
    cand = np.sort(cand, axis=1)
    best = np.empty(b, dtype=np.int64)
    x64 = xf.astype(np.float64)
    step = 256
    for s in range(0, b, step):
        e = min(s + step, b)
        Xc = X_train[cand[s:e]].astype(np.float64)  # [q, C, D]
        diff = x64[s:e, None, :] - Xc
        d2 = np.einsum("qcd,qcd->qc", diff, diff)
        for i in range(e - s):
            mn = d2[i].min()
            best[s + i] = cand[s + i][d2[i] == mn].min()
    return Y_train[best].astype(np.float32)


def kernel(x, X_train, Y_train, _trace=False, _tmpdir=None):
    from concourse.bass_utils import run_bass_kernel_spmd

    x = np.asarray(x, dtype=np.float32)
    X_train = np.asarray(X_train, dtype=np.float32)
    Y_train = np.asarray(Y_train, dtype=np.float32)
    xf = x.reshape(B, D)

    in_maps, perms, biases = _prep_in_maps(xf, X_train)
    nc = _get_nc()
    kw = {}
    if _trace:
        kw = {"trace": True, "tmpdir": _tmpdir}
    res = run_bass_kernel_spmd(nc, in_maps, core_ids=list(range(NCORES)), **kw)

    # host selection: bias, top-K fold-4 columns, expand 4 rows per column
    cands = []
    for c in range(NCORES):
        pooled = np.asarray(res.results[c]["pooled"]).astype(np.float32)  # [B,2048]
        sel = pooled - biases[c][None, :]
        topk = np.argpartition(-sel, TOPK, axis=1)[:, :TOPK]  # [B, K]
        jj, cc = topk // NCHUNK, topk % NCHUNK
        devrows = (
            (np.arange(4)[None, None, :] * 4 + jj[:, :, None]) * NCHUNK
            + cc[:, :, None]
        ).reshape(B, TOPK * 4)
        cands.append(perms[c][devrows] + c * NS)
    cand = np.concatenate(cands, axis=1)  # [B, 8*K*4]
    out = _refine(xf, X_train, Y_train, cand)
    if _trace:
        return out, res
    return out


# revision 8
# speedup vs baseline: 1.2394x; 1.2394x over previous
"""Sharded top-1 KNN (retrieval) on 8 TRN2 NeuronCores via Bass/Tile.

v4 strategy (hardcoded for x[2048,24,16], X_train[65536,384], Y_train[65536,24,1]):
  - Shard X_train rows across 8 cores (8192 rows each).
  - fp8(e4m3) GEMM: cross = x.t with TensorE DoubleRow matmuls (K=256 in one
    MM at ~1 col/cycle) plus a plain fp8 MM for the K=128 tail -> ~1.4x the
    bf16 matmul rate.  Host pre-quantizes x -> [384,2048] fp8 and each
    (tt-sort-permuted) X_train shard -> [384,8192] fp8.
  - Drain: each m-tile's 16 psum chunks (chunk = q*4+j) are max-folded 4->1
    over q into a [128,2048] bf16 fold-4 row using wide ops only: ScalarE
    copies 12 chunks (3 x [128,2048]-ish ACT copies), VectorE merges the
    last 4 psum chunks against one copy and folds the tree (4 wide TTs).
    GpSimd/DMA do no psum work (no PSUM port).  Fold-4 rows DMA to HBM.
  - No bias / top-k on device: the HOST applies the shared -||t||^2/2 bias
    per fold-4 column (4 tt-adjacent rows), takes top-10 columns per core
    (exact, tie-free), expands 4 rows per column, and refines exact
    distances in fp64 (ties -> smallest global index, matching jnp.argmin).
  - Max-pooling cannot hurt candidate recall: the true NN's pooled column
    value >= its own score (measured fold-4 pooled rank <= 2 vs 10 kept).
"""

import os
import sys

import numpy as np

for _p in ("/opt/trn_rl_repo",):
    if os.path.isdir(_p) and _p not in sys.path:
        sys.path.insert(0, _p)

import ml_dtypes  # noqa: E402

B, T, F = 2048, 24, 16
D = T * F  # 384
N = 65536
NCORES = 8
NS = N // NCORES  # 8192 rows per core
MT = B // 128  # 16 query tiles
NCHUNK = 512
NT = NS // NCHUNK  # 16 train chunks per core
NPOOL = 8 * NCHUNK  # 4096 fold-2 pooled positions (g*2048 + j*512 + col)
TOPK = 10  # host-side; fold-4 pooled rank of true NN measured <= 2
NBLK = 8  # X DMA split into 8 column blocks of 1024 for early start
BLKW = NS // NBLK  # 1024

_BF16 = ml_dtypes.bfloat16
_FP8 = ml_dtypes.float8_e4m3


def build_nc(b=B, ns=NS):
    """Build the per-core Bass program (SPMD: same program, per-core inputs)."""
    import concourse.tile as tile
    from concourse import bacc, mybir

    fp8 = mybir.dt.float8e4
    bf16 = mybir.dt.bfloat16
    f32 = mybir.dt.float32
    mx = mybir.AluOpType.max
    DR = mybir.MatmulPerfMode.DoubleRow

    nc = bacc.Bacc(None, target_bir_lowering=False)
    xdr = nc.dram_tensor("xdr", [256, b], fp8, kind="ExternalInput")
    xtl = nc.dram_tensor("xtl", [128, b], fp8, kind="ExternalInput")
    Xdr = nc.dram_tensor("Xdr", [256, ns], fp8, kind="ExternalInput")
    Xtl = nc.dram_tensor("Xtl", [128, ns], fp8, kind="ExternalInput")
    pooled = nc.dram_tensor("pooled", [b, NPOOL], bf16, kind="ExternalOutput")

    with tile.TileContext(nc) as tc:
        with (
            tc.tile_pool(name="wpool", bufs=1) as wpool,
            tc.tile_pool(name="ppool", bufs=1, space="PSUM") as ppool,
            tc.tile_pool(name="spool", bufs=2) as spool,
        ):
            # query weights: DR layout [128, 2, b] (k 0..255) + tail [128, b]
            xw = wpool.tile([128, 2, b], fp8, name="xw", tag="xw")
            nc.sync.dma_start(xw[:, 0, :], xdr[0:128, :])
            nc.sync.dma_start(xw[:, 1, :], xdr[128:256, :])
            xt = wpool.tile([128, b], fp8, name="xt", tag="xt")
            nc.sync.dma_start(xt[:], xtl[:, :])
            # train blocks: 8 col-blocks of 1024, each DR [128,2,1024] + tail
            Xd_b = []
            Xt_b = []
            for blk in range(NBLK):
                cs = slice(blk * BLKW, (blk + 1) * BLKW)
                td = wpool.tile([128, 2, BLKW], fp8, name="Xd", tag=f"Xd{blk}")
                nc.sync.dma_start(td[:, 0, :], Xdr[0:128, cs])
                nc.sync.dma_start(td[:, 1, :], Xdr[128:256, cs])
                Xd_b.append(td)
                tt_ = wpool.tile([128, BLKW], fp8, name="Xt", tag=f"Xt{blk}")
                nc.sync.dma_start(tt_[:], Xtl[:, cs])
                Xt_b.append(tt_)

            def mm_pair(out_ap, m, c):
                ms = slice(m * 128, (m + 1) * 128)
                blk, lo = c // 2, (c % 2) * NCHUNK
                nc.tensor.matmul(
                    out_ap,
                    xw[:, :, ms],
                    Xd_b[blk][:, :, lo : lo + NCHUNK],
                    start=True,
                    stop=False,
                    perf_mode=DR,
                )
                nc.tensor.matmul(
                    out_ap,
                    xt[:, ms],
                    Xt_b[blk][:, lo : lo + NCHUNK],
                    start=False,
                    stop=True,
                )

            for m in range(MT):
                ms = slice(m * 128, (m + 1) * 128)
                # psum: 4 tiles x 2 banks; per group g: A1 = chunks 8g+0,1,
                # A2 = 8g+2,3, B1 = 8g+4,5, B2 = 8g+6,7.  ScalarE copies A1/A2
                # to bf16; VectorE TT-maxes B1/B2 against those copies, giving
                # the fold-2 row x_g[j*512+col] = max over chunks {8g+j, 8g+4+j}
                # which DMAs straight out (no cross-group merge chain on
                # device -- the host folds/biases at fold-2 granularity).
                for g in range(2):
                    A1 = ppool.tile([128, 2, NCHUNK], f32, name="A1", tag="A1")
                    A2 = ppool.tile([128, 2, NCHUNK], f32, name="A2", tag="A2")
                    B1 = ppool.tile([128, 2, NCHUNK], f32, name="B1", tag="B1")
                    B2 = ppool.tile([128, 2, NCHUNK], f32, name="B2", tag="B2")
                    for q, P in enumerate((A1, A2, B1, B2)):
                        for j in range(2):
                            mm_pair(P[:, j, :], m, 8 * g + 2 * q + j)
                    cA = spool.tile([128, 4, NCHUNK], bf16, name="cA")
                    nc.scalar.copy(cA[:, 0:2, :], A1[:])
                    nc.scalar.copy(cA[:, 2:4, :], A2[:])
                    xg = spool.tile([128, 4, NCHUNK], bf16, name="xg")
                    nc.vector.tensor_tensor(xg[:, 0:2, :], B1[:], cA[:, 0:2, :], op=mx)
                    nc.vector.tensor_tensor(xg[:, 2:4, :], B2[:], cA[:, 2:4, :], op=mx)
                    nc.sync.dma_start(
                        pooled[ms, g * 2048 : (g + 1) * 2048], xg[:, :, :]
                    )
    nc.finalize()  # Bacc register allocation; walrus rejects unfinalized BIR
    return nc


_NC = None


def _get_nc():
    global _NC
    if _NC is None:
        _NC = build_nc()
    return _NC


def _shard_perm(tt, ns):
    """Device row n = chunk*512+col, chunk = q*4+j, holds sorted rank
    (col*4+j)*4+q: the 4 rows folded into fold-4 column (j, col) are
    tt-adjacent so one shared bias serves all 4."""
    order = np.argsort(tt, kind="stable")
    n = np.arange(ns)
    chunk, col = n // NCHUNK, n % NCHUNK
    j, q = chunk % 4, chunk // 4
    rank = (col * 4 + j) * 4 + q
    return order[rank]


def _prep_in_maps(xf, X_train):
    xq = np.ascontiguousarray(xf.T).astype(_FP8)  # [384, 2048] fp8
    in_maps = []
    perms = []
    biases = []
    for c in range(NCORES):
        Xs = X_train[c * NS : (c + 1) * NS]
        tt = (Xs.astype(np.float64) ** 2).sum(axis=1)
        perm = _shard_perm(tt, NS)
        perms.append(perm)
        XT = np.ascontiguousarray(Xs[perm].T).astype(_FP8)  # [384, 8192]
        tts = np.sort(tt, kind="stable")
        # fold-2 position g*2048 + j*512 + col pools sorted ranks
        # (col*4+j)*4 + 2g + {0,1}: bias = mean tt/2 of that pair
        pair = tts.reshape(NCHUNK, 4, 2, 2).mean(axis=3) * 0.5  # [col, j, g]
        bias = pair.transpose(2, 1, 0).reshape(NPOOL)  # [g, j, col]
        biases.append(bias.astype(np.float32))
        in_maps.append(
            {
                "xdr": np.ascontiguousarray(xq[0:256]),
                "xtl": np.ascontiguousarray(xq[256:384]),
                "Xdr": np.ascontiguousarray(XT[0:256]),
                "Xtl": np.ascontiguousarray(XT[256:384]),
            }
        )
    return in_maps, perms, biases


def _refine(xf, X_train, Y_train, cand):
    """cand: [B, C] global candidate rows.  fp64 exact distances, ties ->
    smallest global index (matches jnp.argmin first-of-min)."""
    b = cand.shape[0]
    cand = np.sort(cand, axis=1)
    best = np.empty(b, dtype=np.int64)
    x64 = xf.astype(np.float64)
    step = 256
    for s in range(0, b, step):
        e = min(s + step, b)
        Xc = X_train[cand[s:e]].astype(np.float64)  # [q, C, D]
        diff = x64[s:e, None, :] - Xc
        d2 = np.einsum("qcd,qcd->qc", diff, diff)
        for i in range(e - s):
            mn = d2[i].min()
            best[s + i] = cand[s + i][d2[i] == mn].min()
    return Y_train[best].astype(np.float32)


def kernel(x, X_train, Y_train, _trace=False, _tmpdir=None):
    from concourse.bass_utils import run_bass_kernel_spmd

    x = np.asarray(x, dtype=np.float32)
    X_train = np.asarray(X_train, dtype=np.float32)
    Y_train = np.asarray(Y_train, dtype=np.float32)
    xf = x.reshape(B, D)

    in_maps, perms, biases = _prep_in_maps(xf, X_train)
    nc = _get_nc()
    kw = {}
    if _trace:
        kw = {"trace": True, "tmpdir": _tmpdir}
    res = run_bass_kernel_spmd(nc, in_maps, core_ids=list(range(NCORES)), **kw)

    # host selection: bias, top-K fold-2 positions, expand 2 rows per position
    cands = []
    for c in range(NCORES):
        pooled = np.asarray(res.results[c]["pooled"]).astype(np.float32)  # [B,4096]
        sel = pooled - biases[c][None, :]
        topk = np.argpartition(-sel, TOPK, axis=1)[:, :TOPK]  # [B, K]
        gg = topk // 2048
        jj = (topk % 2048) // NCHUNK
        cc = topk % NCHUNK
        # position (g,j,col) pools device rows ((2g+i)*4+j)*512 + col, i in {0,1}
        devrows = (
            ((2 * gg[:, :, None] + np.arange(2)[None, None, :]) * 4 + jj[:, :, None])
            * NCHUNK
            + cc[:, :, None]
        ).reshape(B, TOPK * 2)
        cands.append(perms[c][devrows] + c * NS)
    cand = np.concatenate(cands, axis=1)  # [B, 8*K*2]
    out = _refine(xf, X_train, Y_train, cand)
    if _trace:
        return out, res
    return out
